# revision 6
# baseline (speedup 1.0000x reference)
"""Trainium2 Bass kernel for sliding-window (window=256) causal attention.

Model (B=1, S=4096, H=1024, nh=16, hd=64, no q-scaling):
  q,k,v = x@wq.T, x@wk.T, x@wv.T ; scores = q@k.T (banded causal window 256)
  out = softmax(scores)@v reassembled, then @wo.T + bo

Sharding: 2 heads per core across 8 cores (tensor parallel on the head dim).
Each core computes a partial output  ctx_c @ wo[:, c-slice].T  (+ bias on
core 0 only); the host sums the 8 partials (the all-reduce step).

Per-core pipeline (everything in "transposed" layouts to keep the PE fed):
  P1: xT tiles via PE transpose-mode; QT/KT/VT = w.T.T @ xT  (fp32r matmuls)
  P1.5: V tiles via PE transpose of VT; stored bf16 as [V_h0 | 0 | V_h1]
  P2: per key-tile kt: scoresT = K@Q.T for q-window of 3 tiles (fp32r),
      band mask added on the PE (identity-matmul accumulate of a mask tile),
      exp on ACT (psum -> bf16 sbuf);
      ctx/denominator accumulated per q-tile via zero-padded stationaries so
      both heads + both quantities land partition-aligned in one psum bank;
      normalize with one reciprocal + one multiply; out-projection (fp32r)
      + bias add; DMA out.
"""

import numpy as np

import concourse.bass as bass
import concourse.tile as tile
from concourse import bacc, mybir
from concourse.bass_utils import run_bass_kernel_spmd

S = 4096
H = 1024
NH = 16
HD = 64
WIN = 256
N_CORES = 8
HEADS_PER_CORE = NH // N_CORES  # 2
CD = HEADS_PER_CORE * HD  # 128 ctx dims per core
NEG = -1e30

F32 = mybir.dt.float32
F32R = mybir.dt.float32r
BF16 = mybir.dt.bfloat16

N_ST = S // 128  # 32 s-tiles
N_KT = H // 128  # 8 contraction tiles for projections
N_SC = S // 512  # 8 s-chunks for projections


def _r(ap):
    return ap.bitcast(F32R)


def build_program(taps=False):
    nc = bacc.Bacc("TRN2", target_bir_lowering=False, debug=False)
    tap_aps = {}
    if taps:
        for nm, shp in (("qt_d", [128, S]), ("kt_d", [128, S]), ("vt_d", [128, S]),
                        ("ex_d", [128, 384]), ("sps_d", [128, 384]),
                        ("ctx_d", [128, 128]), ("stg_d", [128, 256])):
            tap_aps[nm] = nc.dram_tensor(nm, shp, F32, kind="ExternalOutput").ap()

    x_ap = nc.dram_tensor("x", [S, H], F32R, kind="ExternalInput").ap()
    wqT_ap = nc.dram_tensor("wqT", [H, CD], F32R, kind="ExternalInput").ap()
    wkT_ap = nc.dram_tensor("wkT", [H, CD], F32R, kind="ExternalInput").ap()
    wvT_ap = nc.dram_tensor("wvT", [H, CD], F32R, kind="ExternalInput").ap()
    woT_ap = nc.dram_tensor("woT", [CD, H], F32R, kind="ExternalInput").ap()
    bo_ap = nc.dram_tensor("bo_b", [128, H], F32, kind="ExternalInput").ap()
    m3_ap = nc.dram_tensor("m3", [128, 384], F32R, kind="ExternalInput").ap()
    id_ap = nc.dram_tensor("ident", [128, 128], F32R, kind="ExternalInput").ap()
    out_ap = nc.dram_tensor("out", [S, H], F32, kind="ExternalOutput").ap()

    with tile.TileContext(nc) as tc:
        with (
            tc.tile_pool(name="consts", bufs=1) as consts,
            tc.tile_pool(name="big", bufs=1) as big,
        ):
            # ---- constant loads ----
            wq_sb = consts.tile([128, N_KT, CD], F32R)
            wk_sb = consts.tile([128, N_KT, CD], F32R)
            wv_sb = consts.tile([128, N_KT, CD], F32R)
            for w_sb, w_ap in ((wq_sb, wqT_ap), (wk_sb, wkT_ap), (wv_sb, wvT_ap)):
                nc.sync.dma_start(
                    out=w_sb[:], in_=w_ap.rearrange("(kt p) d -> p kt d", p=128)
                )
            wo_sb = consts.tile([128, H], F32R)
            nc.sync.dma_start(out=wo_sb[:], in_=woT_ap[:])
            bo_sb = consts.tile([128, H], F32)
            nc.sync.dma_start(out=bo_sb[:], in_=bo_ap[:])
            m3_sb = consts.tile([128, 384], F32R)
            nc.sync.dma_start(out=m3_sb[:], in_=m3_ap[:])
            id_sb = consts.tile([128, 128], F32R)
            nc.sync.dma_start(out=id_sb[:], in_=id_ap[:])
            # dn stationaries: [ones | zeros | ones]; h0 -> [0:128], h1 -> [64:192]
            on2 = consts.tile([128, 192], BF16)
            nc.gpsimd.memset(on2[:, 0:64], 1.0)
            nc.gpsimd.memset(on2[:, 64:128], 0.0)
            nc.gpsimd.memset(on2[:, 128:192], 1.0)

            # ---- persistent activations ----
            qt_sb = big.tile([128, S], F32R)  # QT: [2h*64 dims, S]
            kt_sb = big.tile([128, S], F32R)
            vt_sb = big.tile([128, S], F32R)
            # VA: per key-tile [V_h0(64) | zeros(64) | V_h1(64)] in bf16
            va = big.tile([128, N_ST, 192], BF16)
            nc.gpsimd.memset(va[:, :, 64:128], 0.0)
            # normalized ctx for all q-tiles (consumed by phase-3 out-proj)
            ctx_all = big.tile([128, N_ST, 128], F32R)

            # ================= Phase 1: xT + projections =================
            with (
                tc.tile_pool(name="xstage", bufs=3) as xstage,
                tc.tile_pool(name="xtc", bufs=2) as xtc,
                tc.tile_pool(name="p1t", bufs=3, space="PSUM") as p1t,
                tc.tile_pool(name="p1p", bufs=2, space="PSUM") as p1p,
            ):
                cp_i = 0
                for sc in range(N_SC):
                    xT_c = xtc.tile([128, N_KT, 512], F32R)
                    for st4 in range(4):
                        xst = xstage.tile([128, H], F32R)
                        row0 = (sc * 4 + st4) * 128
                        nc.sync.dma_start(out=xst[:], in_=x_ap[row0 : row0 + 128, :])
                        for kt in range(N_KT):
                            tp = p1t.tile([128, 128], F32R)
                            nc.tensor.transpose(
                                tp[:], xst[:, kt * 128 : (kt + 1) * 128], id_sb[:]
                            )
                            dst = xT_c[:, kt, st4 * 128 : (st4 + 1) * 128]
                            if cp_i % 2 == 0:
                                nc.vector.tensor_copy(dst, tp[:])
                            else:
                                nc.scalar.copy(dst, tp[:])
                            cp_i += 1
                    for w_sb, dstT in ((wq_sb, qt_sb), (wk_sb, kt_sb), (wv_sb, vt_sb)):
                        pps = p1p.tile([128, 512], F32)
                        for kt in range(N_KT):
                            nc.tensor.matmul(
                                pps[:],
                                w_sb[:, kt, :],
                                xT_c[:, kt, :],
                                start=(kt == 0),
                                stop=(kt == N_KT - 1),
                            )
                        dst = dstT[:, sc * 512 : (sc + 1) * 512]
                        if cp_i % 2 == 0:
                            nc.vector.tensor_copy(dst, pps[:])
                        else:
                            nc.scalar.copy(dst, pps[:])
                        cp_i += 1
                if taps:
                    for nm, t in (("qt_d", qt_sb), ("kt_d", kt_sb), ("vt_d", vt_sb)):
                        stg_t = xstage.tile([128, H], F32)
                        for scc in range(4):
                            nc.vector.tensor_copy(stg_t[:], t[:, scc*1024:(scc+1)*1024].bitcast(F32))
                            nc.sync.dma_start(out=tap_aps[nm][:, scc*1024:(scc+1)*1024], in_=stg_t[:])
                # ---- Phase 1.5: V tiles (transpose VT), bf16 ----
                for kt in range(N_ST):
                    tp = p1t.tile([128, 128], F32R)
                    nc.tensor.transpose(
                        tp[:], vt_sb[:, kt * 128 : (kt + 1) * 128], id_sb[:]
                    )
                    nc.vector.tensor_copy(va[:, kt, 0:64], tp[:, 0:64])
                    nc.scalar.copy(va[:, kt, 128:192], tp[:, 64:128])

            # ================= Phase 2: attention + out-proj =================
            with (
                tc.tile_pool(name="p2s", bufs=2, space="PSUM") as p2s,
                tc.tile_pool(name="p2c", bufs=3, space="PSUM") as p2c,
                tc.tile_pool(name="p2d", bufs=3, space="PSUM") as p2d,
                tc.tile_pool(name="expp", bufs=4) as expp,
                tc.tile_pool(name="stgp", bufs=2) as stgp,
                tc.tile_pool(name="recp", bufs=2) as recp,
            ):
                ctx_ps = {}
                dn_ps = {}
                for kt in range(N_ST):
                    W = min(384, (N_ST - kt) * 128)
                    q0 = kt * 128
                    # scoresT for both heads, then PE mask-add, then exp
                    sps_l = []
                    for h in (0, 1):
                        sps = p2s.tile([128, 384], F32)
                        nc.tensor.matmul(
                            sps[:, :W],
                            kt_sb[h * 64 : (h + 1) * 64, q0 : q0 + 128],
                            qt_sb[h * 64 : (h + 1) * 64, q0 : q0 + W],
                            start=True,
                            stop=False,
                        )
                        sps_l.append(sps)
                    for h in (0, 1):
                        nc.tensor.matmul(
                            sps_l[h][:, :W],
                            id_sb[:],
                            m3_sb[:, :W],
                            start=False,
                            stop=True,
                        )
                    ex_l = []
                    for h in (0, 1):
                        ex = expp.tile([128, 384], BF16)
                        nc.scalar.activation(
                            ex[:, :W],
                            sps_l[h][:, :W],
                            mybir.ActivationFunctionType.Exp,
                        )
                        ex_l.append(ex)
                    if taps and kt == 5:
                        tpt = stgp.tile([128, 384], F32)
                        nc.vector.tensor_copy(tpt[:], sps_l[0][:])
                        nc.sync.dma_start(out=tap_aps["sps_d"][:], in_=tpt[:])
                        tpt2 = stgp.tile([128, 384], F32)
                        nc.vector.tensor_copy(tpt2[:], ex_l[0][:])
                        nc.sync.dma_start(out=tap_aps["ex_d"][:], in_=tpt2[:])
                    # ctx + denominator accumulation per q-tile
                    for h in (0, 1):
                        va_h = va[:, kt, 0:128] if h == 0 else va[:, kt, 64:192]
                        on_h = on2[:, 0:128] if h == 0 else on2[:, 64:192]
                        for j in range(W // 128):
                            qt = kt + j
                            if qt not in ctx_ps:
                                ctile = p2c.tile([128, 128], F32)
                                ctx_ps[qt] = ctile
                                dtile = p2d.tile([128, 128], F32)
                                dn_ps[qt] = dtile
                            first = kt == max(qt - 2, 0) and h == 0
                            last = kt == qt and h == 1
                            rhs = ex_l[h][:, j * 128 : (j + 1) * 128]
                            nc.tensor.matmul(
                                ctx_ps[qt][:], va_h, rhs, start=first, stop=last
                            )
                            nc.tensor.matmul(
                                dn_ps[qt][:], on_h, rhs, start=first, stop=last
                            )
                    # finalize q-tile kt: normalize into ctx_all
                    qt = kt
                    stg = stgp.tile([128, 256], F32)
                    nc.scalar.copy(stg[:, 0:128], ctx_ps.pop(qt)[:])
                    nc.scalar.copy(stg[:, 128:256], dn_ps.pop(qt)[:])
                    rec = recp.tile([128, 128], F32)
                    nc.vector.reciprocal(rec[:], stg[:, 128:256])
                    nc.vector.tensor_mul(ctx_all[:, qt, :], stg[:, 0:128], rec[:])
                    if taps and qt == 5:
                        nc.sync.dma_start(out=tap_aps["ctx_d"][:], in_=ctx_all[:, qt, :].bitcast(F32))
                        nc.sync.dma_start(out=tap_aps["stg_d"][:], in_=stg[:])

            # ============== Phase 3: out-projection + bias + store ==============
            with (
                tc.tile_pool(name="p3o", bufs=4, space="PSUM") as p3o,
                tc.tile_pool(name="outp", bufs=3) as outp,
            ):
                for qt in range(N_ST):
                    osb = outp.tile([128, H], F32)
                    for half in range(2):
                        ops = p3o.tile([128, 512], F32)
                        nc.tensor.matmul(
                            ops[:],
                            ctx_all[:, qt, :],
                            wo_sb[:, half * 512 : (half + 1) * 512],
                            start=True,
                            stop=True,
                        )
                        nc.vector.tensor_add(
                            osb[:, half * 512 : (half + 1) * 512],
                            ops[:],
                            bo_sb[:, half * 512 : (half + 1) * 512],
                        )
                    nc.sync.dma_start(
                        out=out_ap[qt * 128 : (qt + 1) * 128, :], in_=osb[:]
                    )

    nc.compile()
    return nc


def build_in_maps(x, wq, wk, wv, wo, bo):
    xf = np.ascontiguousarray(x.reshape(S, H), dtype=np.float32)

    # band mask blocks in [k-part, q-free] tile coords, additive
    b = np.arange(128)[:, None]
    a = np.arange(128)[None, :]
    mask_a = np.where(b <= a, 0.0, NEG).astype(np.float32)  # diag tile (qt==kt)
    mask_b = np.where(b > a, 0.0, NEG).astype(np.float32)  # qt==kt+2 tile
    m3 = np.concatenate(
        [mask_a, np.zeros((128, 128), np.float32), mask_b], axis=1
    )
    ident = np.eye(128, dtype=np.float32)

    in_maps = []
    for c in range(N_CORES):
        r0, r1 = c * CD, (c + 1) * CD
        bo_b = np.broadcast_to(
            (bo if c == 0 else np.zeros_like(bo)).astype(np.float32), (128, H)
        ).copy()
        in_maps.append(
            {
                "x": xf,
                "wqT": np.ascontiguousarray(wq[r0:r1, :].T, dtype=np.float32),
                "wkT": np.ascontiguousarray(wk[r0:r1, :].T, dtype=np.float32),
                "wvT": np.ascontiguousarray(wv[r0:r1, :].T, dtype=np.float32),
                "woT": np.ascontiguousarray(wo[:, r0:r1].T, dtype=np.float32),
                "bo_b": bo_b,
                "m3": m3,
                "ident": ident,
            }
        )
    return in_maps


_NC_CACHE = None


def kernel(x, wq, wk, wv, wo, bo):
    global _NC_CACHE
    if _NC_CACHE is None:
        _NC_CACHE = build_program()
    nc = _NC_CACHE
    in_maps = build_in_maps(x, wq, wk, wv, wo, bo)
    res = run_bass_kernel_spmd(nc, in_maps, list(range(N_CORES)))
    out = res.results[0]["out"].astype(np.float64)
    for c in range(1, N_CORES):
        out += res.results[c]["out"]
    return out.reshape(1, S, H).astype(np.float32)
